# revision 1
# baseline (speedup 1.0000x reference)
"""Trainium2 Bass kernel for nn_AttentionProjector (8-core SPMD).

Math: out = softmax(q @ (x@Wk.T).T / ...) @ (x@Wv.T + Wv_b)
Rewritten to avoid materializing keys/values:
    scores = (q @ Wk) @ x.T          (+ q@Wk_b, constant per row -> cancels in softmax)
    out    = (softmax(scores) @ x) @ Wv.T + Wv_b
This cuts FLOPs from ~600G to ~52G. All matmuls run in float32r (FP22
mantissa, full PE rate at free-dim>=256).

Sharding (8 cores):
  phase 1: q'T = Wk.T @ q.T          -- contraction (dout) sharded, AllReduce-add
  phase 2: scores[l, n_j]            -- token dim N sharded (1024/core)
  softmax: local max -> AllReduce-max -> p = exp(scores - M), s_j partial sums
  phase 3: uT_j = x_j.T @ p_j.T      -- local weighted sums, AllReduce-add (u, s)
  phase 4: out[:, do_j] = (uT/S).T @ Wv[do_j,:].T + Wv_b[do_j]  -- dout sharded

Inputs are fed per-core as pre-sliced / pre-transposed arrays (host-side
sharding); output slices are concatenated on host.
"""

import numpy as np

L = 256          # query rows
D = 4096         # d_in == d_out
N = 8192         # tokens
NCORES = 8
NS = N // NCORES     # 1024 tokens per core
DS = D // NCORES     # 512 dout per core

LT = L // 128        # 2 l-tiles
DT = D // 128        # 32 d-tiles
NT = NS // 128       # 8 local n-tiles

_MAX_WAITS = 1


def _split_waits(nc, mybir, bass_rust):
    """Walrus in this container allows only one sync-wait per instruction;
    move excess waits onto preceding same-engine no-ops."""
    for bb in nc.main_func.blocks:
        new_list = []
        for ins in bb.instructions:
            si = ins.sync_info
            waits = list(si.on_wait) if si is not None else []
            if len(waits) > _MAX_WAITS:
                for i in range(_MAX_WAITS, len(waits), _MAX_WAITS):
                    nop = mybir.InstNoOp(name=f"{ins.name}-wsplit{i}", ins=[], outs=[])
                    nop.engine = ins.engine
                    nop.sync_info = bass_rust.SyncInfo(
                        on_wait=waits[i:i + _MAX_WAITS], on_update=[])
                    new_list.append(nop)
                ins.sync_info = bass_rust.SyncInfo(
                    on_wait=waits[:_MAX_WAITS], on_update=si.on_update)
            new_list.append(ins)
        bb.instructions[:] = new_list


_NC = None


def _build(split_waits=True):
    global _NC
    if _NC is not None and split_waits:
        return _NC
    import bass_rust
    import concourse.bass as bass
    import concourse.mybir as mybir
    import concourse.tile as tile
    from concourse.masks import make_identity
    from contextlib import ExitStack

    f32 = mybir.dt.float32
    f32r = mybir.dt.float32r
    bf16 = mybir.dt.bfloat16
    AF = mybir.ActivationFunctionType
    AX = mybir.AxisListType
    ALU = mybir.AluOpType
    RG = [list(range(NCORES))]

    nc = bass.Bass()

    # per-core external I/O
    t_qts = nc.dram_tensor("qTs", [D, L], f32r, kind="ExternalInput")
    t_wk = nc.dram_tensor("wk", [D, DS], f32r, kind="ExternalInput")
    t_x = nc.dram_tensor("x", [NS, D], f32r, kind="ExternalInput")
    t_xt = nc.dram_tensor("xT", [D, NS], f32r, kind="ExternalInput")
    t_wvt = nc.dram_tensor("wvT", [D, DS], f32r, kind="ExternalInput")
    t_wvb = nc.dram_tensor("wvb", [1, DS], f32, kind="ExternalInput")
    t_out = nc.dram_tensor("out", [L, DS], f32, kind="ExternalOutput")

    # collective bounce buffers (input Local, output Shared)
    ar_q_in = nc.dram_tensor("ar_q_in", [DS, L], f32)
    ar_q_out = nc.dram_tensor("ar_q_out", [D, L], f32, addr_space="Shared")
    ar_m_in = nc.dram_tensor("ar_m_in", [L, 1], f32)
    ar_m_out = nc.dram_tensor("ar_m_out", [L, 1], f32, addr_space="Shared")
    ar_s_in = nc.dram_tensor("ar_s_in", [L, 1], f32)
    ar_s_out = nc.dram_tensor("ar_s_out", [L, 1], f32, addr_space="Shared")
    NQ = 4
    ar_u_in = [nc.dram_tensor(f"ar_u_in{h}", [D // NQ, L], f32) for h in range(NQ)]
    ar_u_out = [nc.dram_tensor(f"ar_u_out{h}", [D // NQ, L], f32, addr_space="Shared")
                for h in range(NQ)]

    qts_re = t_qts.ap().rearrange("(kt p) l -> p kt l", p=128)   # [128, 32, 256]
    wk_re = t_wk.ap().rearrange("(kt p) d -> p kt d", p=128)     # [128, 32, 512]
    x_re = t_x.ap().rearrange("(nt p) d -> p nt d", p=128)       # [128, 8, 4096]
    xt_re = t_xt.ap().rearrange("(dt p) n -> p dt n", p=128)     # [128, 32, 1024]
    wvt_re = t_wvt.ap().rearrange("(dt p) o -> p dt o", p=128)   # [128, 32, 512]
    arq_re = ar_q_in.ap().rearrange("(dt p) l -> p dt l", p=128)
    arqo_re = ar_q_out.ap().rearrange("(dt p) l -> p dt l", p=128)
    aru_re = [t.ap().rearrange("(dt p) l -> p dt l", p=128) for t in ar_u_in]
    aruo_re = [t.ap().rearrange("(dt p) l -> p dt l", p=128) for t in ar_u_out]

    with ExitStack() as ctx:
        tc = ctx.enter_context(tile.TileContext(nc))
        const = ctx.enter_context(tc.tile_pool(name="const", bufs=1))
        small = ctx.enter_context(tc.tile_pool(name="small", bufs=1))
        persist = ctx.enter_context(tc.tile_pool(name="persist", bufs=1))

        # persistent across phases
        pT = persist.tile([128, NT, L], f32r)       # p.T (1MB)

        # ---------------- phase 1: q'T partial = Wk[do_j].T @ q.T[do_j] ------
        with tc.tile_pool(name="ph1", bufs=1) as ph1, \
             tc.tile_pool(name="ph1wk", bufs=2) as ph1wk, \
             tc.tile_pool(name="ph1ps", bufs=1, space="PSUM") as ph1ps:
            qts_sb = ph1.tile([128, 32, L], f32r)    # full q.T
            for kc in range(4):
                nc.sync.dma_start(qts_sb[:, kc * 8:(kc + 1) * 8, :],
                                  qts_re[:, kc * 8:(kc + 1) * 8, :])
            ident = const.tile([128, 128], f32)
            make_identity(nc, ident[:])
            bias_sb = const.tile([128, DS], f32)
            nc.scalar.dma_start(bias_sb[:],
                                t_wvb.ap().partition_broadcast(128)[:, 0, :])
            qpT_loc = ph1.tile([128, 4, L], f32)     # local di slice of q'T
            ps4 = [ph1ps.tile([128, L], f32, name=f"ph1ps{i}") for i in range(4)]
            KCH = 8                                  # k-tiles per wk chunk (2MB)
            for kc in range(32 // KCH):
                wk_c = ph1wk.tile([128, KCH, DS], f32r, name="wk_c")
                nc.sync.dma_start(wk_c[:], wk_re[:, kc * KCH:(kc + 1) * KCH, :])
                for i in range(KCH):
                    kt = kc * KCH + i
                    for dtl in range(4):
                        nc.tensor.matmul(
                            ps4[dtl][:], wk_c[:, i, dtl * 128:(dtl + 1) * 128],
                            qts_sb[:, kt, :], start=(kt == 0), stop=(kt == 31))
            for dtl in range(4):
                nc.vector.tensor_copy(qpT_loc[:, dtl, :], ps4[dtl][:])
            nc.gpsimd.dma_start(arq_re, qpT_loc[:])
            nc.gpsimd.collective_compute(
                "AllGather", ALU.bypass, replica_groups=RG,
                ins=[ar_q_in.ap().opt()], outs=[ar_q_out.ap().opt()])
        # read back with f32 -> f32r rounding (SWDGE cast dma), in chunks so
        # phase 2's first matmuls can start before the whole 4.2MB lands
        qpT = persist.tile([128, DT, L], f32r, name="qpT")
        for rc in range(4):
            nc.gpsimd.dma_start(qpT[:, rc * 8:(rc + 1) * 8, :],
                                arqo_re[:, rc * 8:(rc + 1) * 8, :])

        # ---------------- phase 2: scores[l, n_j], streaming xT --------------
        XCH = 8                      # d-tiles per xT chunk (4MB)
        with tc.tile_pool(name="ph2sc", bufs=1, space="PSUM") as scps_pool, \
             tc.tile_pool(name="ph2xt", bufs=2) as xt_pool:
            score_ps = [[scps_pool.tile([128, 512], f32, name=f'score{i}_{k}')
                         for k in range(2)] for i in range(LT)]
            for c in range(DT // XCH):
                xt_c = xt_pool.tile([128, XCH, NS], f32r)
                nc.sync.dma_start(xt_c[:], xt_re[:, c * XCH:(c + 1) * XCH, :])
                for i in range(XCH):
                    dt = c * XCH + i
                    for lt in range(LT):
                        for nch in range(2):
                            nc.tensor.matmul(
                                score_ps[lt][nch][:],
                                qpT[:, dt, lt * 128:(lt + 1) * 128],
                                xt_c[:, i, nch * 512:(nch + 1) * 512],
                                start=(dt == 0), stop=(dt == DT - 1))

            # local row max -> AllReduce max (both l-tiles packed in one DMA)
            m_both = small.tile([128, 2], f32, name="m_both")
            for lt in range(LT):
                mtmp = small.tile([128, 1], f32, name=f"mtmp{lt}")
                nc.vector.tensor_reduce(mtmp[:], score_ps[lt][0][:], axis=AX.X, op=ALU.max)
                nc.vector.tensor_reduce(m_both[:, lt:lt + 1], score_ps[lt][1][:],
                                        axis=AX.X, op=ALU.max)
                nc.vector.tensor_tensor(m_both[:, lt:lt + 1], m_both[:, lt:lt + 1],
                                        mtmp[:], ALU.max)
            nc.gpsimd.dma_start(
                ar_m_in.ap().rearrange("(lt p) o -> p (lt o)", p=128), m_both[:])
            nc.gpsimd.collective_compute(
                "AllReduce", ALU.max, replica_groups=RG,
                ins=[ar_m_in.ap().opt()], outs=[ar_m_out.ap().opt()])

            # p = exp(scores - M), s_j = row sums
            p_sb = [persist.tile([128, NS], f32, name=f'p_sb{i}') for i in range(LT)]
            msb2 = small.tile([128, 2], f32, name="msb2")
            nc.gpsimd.dma_start(
                msb2[:], ar_m_out.ap().rearrange("(lt p) o -> p (lt o)", p=128))
            negM2 = small.tile([128, 2], f32, name="negM2")
            nc.vector.tensor_scalar_mul(negM2[:], msb2[:], -1.0)
            s_both = small.tile([128, 2], f32, name="s_both")
            for lt in range(LT):
                sp0 = small.tile([128, 1], f32, name=f"sp0_{lt}")
                nc.scalar.activation(p_sb[lt][:, 0:512], score_ps[lt][0][:],
                                     AF.Exp, bias=negM2[:, lt:lt + 1], accum_out=sp0[:])
                nc.scalar.activation(p_sb[lt][:, 512:1024], score_ps[lt][1][:],
                                     AF.Exp, bias=negM2[:, lt:lt + 1],
                                     accum_out=s_both[:, lt:lt + 1])
                nc.vector.tensor_tensor(s_both[:, lt:lt + 1], s_both[:, lt:lt + 1],
                                        sp0[:], ALU.add)
            nc.gpsimd.dma_start(
                ar_s_in.ap().rearrange("(lt p) o -> p (lt o)", p=128), s_both[:])
            nc.gpsimd.collective_compute(
                "AllReduce", ALU.add, replica_groups=RG,
                ins=[ar_s_in.ap().opt()], outs=[ar_s_out.ap().opt()])

        # ---------------- transpose p -> pT [n, l] ---------------------------
        with tc.tile_pool(name="tp", bufs=2, space="PSUM") as tpps:
            for lt in range(LT):
                for nt in range(NT):
                    tp = tpps.tile([128, 128], f32)
                    nc.tensor.transpose(
                        tp[:], p_sb[lt][:, nt * 128:(nt + 1) * 128], ident[:])
                    nc.vector.tensor_copy(pT[:, nt, lt * 128:(lt + 1) * 128], tp[:])

        # ---- phase 3: uT = x_j.T @ p.T, streaming x; AR-u split in halves ---
        XCH3 = 4                     # d-tiles per x chunk (2MB)
        HT = DT // NQ                # d-tiles per AR-u quarter
        uT_h = [None] * NQ
        ctxT_h = [None] * NQ
        with tc.tile_pool(name="ph3x", bufs=2) as x_pool, \
             tc.tile_pool(name="ph3ps", bufs=2, space="PSUM") as ph3ps:
            for h in range(NQ):
                uT_h[h] = persist.tile([128, HT, L], f32, name=f"uT_h{h}")
                for c in range(max(1, HT // XCH3)):
                    base = h * HT + c * XCH3
                    nch3 = min(XCH3, HT)
                    x_c = x_pool.tile([128, NT, XCH3 * 128], f32r)
                    nc.sync.dma_start(
                        x_c[:, :, :nch3 * 128],
                        x_re[:, :, base * 128:(base + nch3) * 128])
                    for i in range(nch3):
                        dt = base + i
                        psu = ph3ps.tile([128, L], f32)
                        for nt in range(NT):
                            nc.tensor.matmul(
                                psu[:], x_c[:, nt, i * 128:(i + 1) * 128],
                                pT[:, nt, :], start=(nt == 0), stop=(nt == NT - 1))
                        nc.vector.tensor_copy(uT_h[h][:, dt - h * HT, :], psu[:])
                nc.gpsimd.dma_start(aru_re[h], uT_h[h][:])
                nc.gpsimd.collective_compute(
                    "AllReduce", ALU.add, replica_groups=RG,
                    ins=[ar_u_in[h].ap().opt()], outs=[ar_u_out[h].ap().opt()])
                ctxT_h[h] = persist.tile([128, HT, L], f32r, name=f"ctxT_h{h}")
                nc.gpsimd.dma_start(ctxT_h[h][:], aruo_re[h])   # f32 -> f32r

        # 1/S
        ssb2 = small.tile([128, 2], f32, name="ssb2")
        nc.gpsimd.dma_start(
            ssb2[:], ar_s_out.ap().rearrange("(lt p) o -> p (lt o)", p=128))
        rec2 = small.tile([128, 2], f32, name="rec2")
        nc.vector.reciprocal(rec2[:], ssb2[:])

        # ---------------- phase 4: out = (ctxT/S).T @ WvT + Wv_b -------------
        WCH = 8                      # d-tiles per wvT chunk (2MB)
        with tc.tile_pool(name="ph4w", bufs=2) as wv_pool, \
             tc.tile_pool(name="ph4ps", bufs=1, space="PSUM") as ph4ps, \
             tc.tile_pool(name="ph4o", bufs=2) as out_pool:
            po = [ph4ps.tile([128, DS], f32, name=f'po{i}') for i in range(LT)]
            for c in range(DT // WCH):
                wv_c = wv_pool.tile([128, WCH, DS], f32r)
                nc.sync.dma_start(wv_c[:], wvt_re[:, c * WCH:(c + 1) * WCH, :])
                for i in range(WCH):
                    dt = c * WCH + i
                    ct = ctxT_h[dt // HT]
                    for lt in range(LT):
                        nc.tensor.matmul(
                            po[lt][:], ct[:, dt % HT, lt * 128:(lt + 1) * 128],
                            wv_c[:, i, :], start=(dt == 0), stop=(dt == DT - 1))
            for lt in range(LT):
                o_sb = out_pool.tile([128, DS], f32)
                nc.scalar.activation(o_sb[:], po[lt][:], AF.Copy,
                                     scale=rec2[:, lt:lt + 1])
                nc.vector.tensor_tensor(o_sb[:], o_sb[:], bias_sb[:], ALU.add)
                nc.gpsimd.dma_start(t_out[lt * 128:(lt + 1) * 128, :], o_sb[:])

    if split_waits:
        _split_waits(nc, mybir, bass_rust)
        _NC = nc
    return nc


last_results = None


def kernel(src_prompts, query, Wk_w, Wk_b, Wv_w, Wv_b):
    global last_results
    from concourse.bass_utils import run_bass_kernel_spmd

    nc = _build()

    x = np.ascontiguousarray(np.asarray(src_prompts, dtype=np.float32)[0])
    q = np.asarray(query, dtype=np.float32)
    wk = np.asarray(Wk_w, dtype=np.float32)
    wv = np.asarray(Wv_w, dtype=np.float32)
    wvb = np.asarray(Wv_b, dtype=np.float32)
    # Wk_b shifts every score row by a constant -> cancels in softmax.

    qT = np.ascontiguousarray(q.T)
    in_maps = []
    for j in range(NCORES):
        ns, ds = slice(j * NS, (j + 1) * NS), slice(j * DS, (j + 1) * DS)
        xj = x[ns]
        in_maps.append({
            "qTs": qT,
            "wk": np.ascontiguousarray(wk[:, ds]),
            "x": np.ascontiguousarray(xj),
            "xT": np.ascontiguousarray(xj.T),
            "wvT": np.ascontiguousarray(wv[ds].T),
            "wvb": np.ascontiguousarray(wvb[ds][None, :]),
        })

    res = run_bass_kernel_spmd(nc, in_maps, core_ids=list(range(NCORES)))
    last_results = res
    out = np.concatenate([res.results[j]["out"] for j in range(NCORES)], axis=1)
    return out[None, :, :]



# revision 7
# speedup vs baseline: 1.4856x; 1.4856x over previous
"""Trainium2 Bass kernel for nn_AttentionProjector (8-core SPMD), v2.

Math: out = softmax(q @ (x@Wk.T).T) @ (x@Wv.T) + Wv_b
Rewritten (Wk_b cancels in softmax):
    q'     = q @ Wk                    [L, D]
    scores = q' @ x.T                  [L, N]
    out    = (softmax(scores) @ x) @ Wv.T + Wv_b
~52 GFLOP total, 6.45 GFLOP/core. All matmuls fp16 (f32 accumulate);
fp16 gives full PE rate, half the HBM traffic of f32, and enough
mantissa (10 bits) for the near-one-hot softmax (host-validated
rel err ~5e-3 vs 2e-2 budget).

Sharding (8 cores) - all collectives are AllGathers:
  phase 1: q'T slice [512, L] per core (Wk cols sharded) -> AG-q'
  phase 2: scores[l, n_j] (token dim sharded), xT streamed
  softmax: exp with LOCAL max immediately (no global-max wait);
           AG of (m_loc, s_loc) packed as [4,128] rows; every core then
           derives global M, rescales its own p by exp(m_loc-M) (folded
           into the transpose matmul via a diagonal matrix), and computes
           S = sum_j s_j*exp(m_j-M) locally.
  phase 3: uT[ds_j, L] FULL contraction over n using a column slice
           x[:, ds_j] (third host layout of x) and the AG'd pT -> no
           AllReduce needed -> AG-uT
  phase 4: out[:, ds_j] = (uT/S).T @ Wv[ds_j,:].T + Wv_b[ds_j]

All HBM inputs are host-pre-tiled to [128, F] with each partition's
bytes contiguous (2-16KB DMA descriptors instead of the 0.5-4KB the
naive rearranges generate). Bulk loads go on the sync HWDGE ring in
phase order (2-buf pools pace it); small bounce writes on scalar;
collectives + readbacks on gpsimd.
"""

import numpy as np

L = 256          # query rows
D = 4096         # d_in == d_out
N = 8192         # tokens
NCORES = 8
NS = N // NCORES     # 1024 tokens per core
DS = D // NCORES     # 512 d-slice per core

LT = L // 128        # 2 l-tiles
DT = D // 128        # 32 d-tiles
NTL = NS // 128      # 8 local n-tiles
NTA = N // 128       # 64 global n-tiles

_MAX_WAITS = 1


def _split_waits(nc, mybir, bass_rust):
    """Walrus in this container allows only one sync-wait per instruction;
    move excess waits onto preceding same-engine no-ops."""
    for bb in nc.main_func.blocks:
        new_list = []
        for ins in bb.instructions:
            si = ins.sync_info
            waits = list(si.on_wait) if si is not None else []
            if len(waits) > _MAX_WAITS:
                for i in range(_MAX_WAITS, len(waits), _MAX_WAITS):
                    nop = mybir.InstNoOp(name=f"{ins.name}-wsplit{i}", ins=[], outs=[])
                    nop.engine = ins.engine
                    nop.sync_info = bass_rust.SyncInfo(
                        on_wait=waits[i:i + _MAX_WAITS], on_update=[])
                    new_list.append(nop)
                ins.sync_info = bass_rust.SyncInfo(
                    on_wait=waits[:_MAX_WAITS], on_update=si.on_update)
            new_list.append(ins)
        bb.instructions[:] = new_list


_NC = None


def _build(split_waits=True):
    global _NC
    if _NC is not None and split_waits:
        return _NC
    import bass_rust
    import concourse.bass as bass
    import concourse.mybir as mybir
    import concourse.tile as tile
    from concourse.masks import make_identity
    from contextlib import ExitStack

    f32 = mybir.dt.float32
    f16 = mybir.dt.float16
    AF = mybir.ActivationFunctionType
    AX = mybir.AxisListType
    ALU = mybir.AluOpType
    RG = [list(range(NCORES))]

    nc = bass.Bass()

    # per-core external I/O (host pre-tiled, see kernel() below)
    t_qts = nc.dram_tensor("qts", [128, DT * L], f16, kind="ExternalInput")
    t_wk = nc.dram_tensor("wk", [128, DT * DS], f16, kind="ExternalInput")
    t_xt = nc.dram_tensor("xt", [128, DT * NS], f16, kind="ExternalInput")
    t_xc = nc.dram_tensor("xc", [128, NTA * DS], f16, kind="ExternalInput")
    t_wvt = nc.dram_tensor("wvt", [128, DT * DS], f16, kind="ExternalInput")
    t_wvb = nc.dram_tensor("wvb", [1, DS], f32, kind="ExternalInput")
    t_out = nc.dram_tensor("out", [L, DS], f32, kind="ExternalOutput")

    # collective bounce buffers (input Local, output Shared)
    agq_i = nc.dram_tensor("agq_i", [128, 4 * L], f16)
    agq_o = nc.dram_tensor("agq_o", [128 * NCORES, 4 * L], f16, addr_space="Shared")
    agms_i = nc.dram_tensor("agms_i", [4, 128], f32)
    agms_o = nc.dram_tensor("agms_o", [4 * NCORES, 128], f32, addr_space="Shared")
    agp_i = nc.dram_tensor("agp_i", [128, NTL * L], f16)
    agp_o = nc.dram_tensor("agp_o", [128 * NCORES, NTL * L], f16, addr_space="Shared")
    agu_i = nc.dram_tensor("agu_i", [128, 4 * L], f16)
    agu_o = nc.dram_tensor("agu_o", [128 * NCORES, 4 * L], f16, addr_space="Shared")

    qts_re = t_qts.ap().rearrange("p (t l) -> p t l", t=DT)     # [128, 32, 256]
    wk_re = t_wk.ap().rearrange("p (t d) -> p t d", t=DT)       # [128, 32, 512]
    xt_re = t_xt.ap().rearrange("p (t n) -> p t n", t=DT)       # [128, 32, 1024]
    xc_re = t_xc.ap().rearrange("p (t d) -> p t d", t=NTA)      # [128, 64, 512]
    wvt_re = t_wvt.ap().rearrange("p (t o) -> p t o", t=DT)     # [128, 32, 512]
    agqo_re = agq_o.ap().rearrange("(r p) (t l) -> p r t l", p=128, t=4)
    agpo_re = agp_o.ap().rearrange("(r p) (t l) -> p r t l", p=128, t=NTL)
    aguo_re = agu_o.ap().rearrange("(r p) (t l) -> p r t l", p=128, t=4)

    with ExitStack() as ctx:
        tc = ctx.enter_context(tile.TileContext(nc))
        const = ctx.enter_context(tc.tile_pool(name="const", bufs=1))
        small = ctx.enter_context(tc.tile_pool(name="small", bufs=1))
        persist = ctx.enter_context(tc.tile_pool(name="persist", bufs=1))

        # constants
        ident16 = const.tile([128, 128], f16)
        make_identity(nc, ident16[:])
        ident32 = const.tile([128, 128], f32)
        make_identity(nc, ident32[:])
        bias_sb = const.tile([128, DS], f32)
        nc.scalar.dma_start(bias_sb[:],
                            t_wvb.ap().partition_broadcast(128)[:, 0, :])

        # persistent tiles
        qpT = persist.tile([128, DT, L], f16, name="qpT")        # full q'T (2MB)
        p_sb = persist.tile([128, LT, NS], f16, name="p_sb")     # local p (0.5MB)
        pT_loc = persist.tile([128, NTL, L], f16, name="pT_loc")
        pT = persist.tile([128, NTA, L], f16, name="pT")         # full pT (4MB)
        uT_loc = persist.tile([128, 4, L], f16, name="uT_loc")
        ctxT = persist.tile([128, DT, L], f16, name="ctxT")      # full uT (2MB)

        # ------------- phase 1: q'T slice = Wk[:, ds_j].T @ q.T --------------
        with tc.tile_pool(name="ph1", bufs=1) as ph1, \
             tc.tile_pool(name="ph1wk", bufs=2) as ph1wk, \
             tc.tile_pool(name="ph1ps", bufs=1, space="PSUM") as ph1ps:
            qts_sb = ph1.tile([128, DT, L], f16)
            for c in range(4):
                nc.sync.dma_start(qts_sb[:, c * 8:(c + 1) * 8, :],
                                  qts_re[:, c * 8:(c + 1) * 8, :])
            ps1 = [ph1ps.tile([128, L], f32, name=f"ps1_{i}") for i in range(4)]
            qpT_l = ph1.tile([128, 4, L], f16)
            for kc in range(4):
                wk_c = ph1wk.tile([128, 8, DS], f16, name="wk_c")
                nc.sync.dma_start(wk_c[:], wk_re[:, kc * 8:(kc + 1) * 8, :])
                for i in range(8):
                    kt = kc * 8 + i
                    for dtl in range(4):
                        nc.tensor.matmul(
                            ps1[dtl][:], wk_c[:, i, dtl * 128:(dtl + 1) * 128],
                            qts_sb[:, kt, :], start=(kt == 0), stop=(kt == 31))
            for dtl in range(4):
                nc.vector.tensor_copy(qpT_l[:, dtl, :], ps1[dtl][:])
            nc.scalar.dma_start(agq_i.ap().rearrange("p (t l) -> p t l", t=4),
                                qpT_l[:])
            nc.gpsimd.collective_compute(
                "AllGather", ALU.bypass, replica_groups=RG,
                ins=[agq_i.ap().opt()], outs=[agq_o.ap().opt()])
            qpT_4d = qpT[:].rearrange("p (r t) l -> p r t l", r=NCORES)
            for c in range(2):
                nc.gpsimd.dma_start(qpT_4d[:, c * 4:(c + 1) * 4, :, :],
                                    agqo_re[:, c * 4:(c + 1) * 4, :, :])

        # ------------- phase 2: scores[l, n_j], streaming xT -----------------
        with tc.tile_pool(name="ph2xt", bufs=2) as ph2xt, \
             tc.tile_pool(name="ph2ps", bufs=1, space="PSUM") as ph2ps:
            score_ps = [[ph2ps.tile([128, 512], f32, name=f"sc{i}_{k}")
                         for k in range(2)] for i in range(LT)]
            for c in range(4):
                xt_c = ph2xt.tile([128, 8, NS], f16, name="xt_c")
                nc.sync.dma_start(xt_c[:], xt_re[:, c * 8:(c + 1) * 8, :])
                for i in range(8):
                    dt = c * 8 + i
                    for lt in range(LT):
                        for nh in range(2):
                            nc.tensor.matmul(
                                score_ps[lt][nh][:],
                                qpT[:, dt, lt * 128:(lt + 1) * 128],
                                xt_c[:, i, nh * 512:(nh + 1) * 512],
                                start=(dt == 0), stop=(dt == DT - 1))

            # local row max; exp with LOCAL max (no global-max wait)
            m_loc = small.tile([128, LT], f32, name="m_loc")
            s_loc = small.tile([128, LT], f32, name="s_loc")
            neg_m = small.tile([128, LT], f32, name="neg_m")
            for lt in range(LT):
                mtmp = small.tile([128, 1], f32, name=f"mtmp{lt}")
                nc.vector.tensor_reduce(mtmp[:], score_ps[lt][0][:],
                                        axis=AX.X, op=ALU.max)
                nc.vector.tensor_reduce(m_loc[:, lt:lt + 1], score_ps[lt][1][:],
                                        axis=AX.X, op=ALU.max)
                nc.vector.tensor_tensor(m_loc[:, lt:lt + 1], m_loc[:, lt:lt + 1],
                                        mtmp[:], ALU.max)
            nc.vector.tensor_scalar_mul(neg_m[:], m_loc[:], -1.0)
            for lt in range(LT):
                sp0 = small.tile([128, 1], f32, name=f"sp0_{lt}")
                nc.scalar.activation(p_sb[:, lt, 0:512], score_ps[lt][0][:],
                                     AF.Exp, bias=neg_m[:, lt:lt + 1],
                                     accum_out=sp0[:])
                nc.scalar.activation(p_sb[:, lt, 512:1024], score_ps[lt][1][:],
                                     AF.Exp, bias=neg_m[:, lt:lt + 1],
                                     accum_out=s_loc[:, lt:lt + 1])
                nc.vector.tensor_tensor(s_loc[:, lt:lt + 1], s_loc[:, lt:lt + 1],
                                        sp0[:], ALU.add)

        # ---- AG of (m_loc, s_loc): transpose to [4,128] rows so the AG ----
        # ---- readback is 512B-contiguous per partition ----
        ms_pack = small.tile([128, 4], f32, name="ms_pack")
        nc.vector.tensor_copy(ms_pack[:, 0:2], m_loc[:])
        nc.vector.tensor_copy(ms_pack[:, 2:4], s_loc[:])
        with tc.tile_pool(name="msps", bufs=1, space="PSUM") as msps:
            ms_tp = msps.tile([4, 128], f32)
            nc.tensor.transpose(ms_tp[:], ms_pack[:], ident32[:])
            ms_tps = small.tile([4, 128], f32, name="ms_tps")
            nc.vector.tensor_copy(ms_tps[:], ms_tp[:])
            nc.scalar.dma_start(agms_i.ap(), ms_tps[:])
            nc.gpsimd.collective_compute(
                "AllGather", ALU.bypass, replica_groups=RG,
                ins=[agms_i.ap().opt()], outs=[agms_o.ap().opt()])
            ms_all_t = small.tile([32, 128], f32, name="ms_all_t")
            nc.gpsimd.dma_start(ms_all_t[:], agms_o.ap())
            ms_tp2 = msps.tile([128, 32], f32)
            nc.tensor.transpose(ms_tp2[:], ms_all_t[:], ident32[0:32, 0:32])
            ms_all = small.tile([128, 8, 4], f32, name="ms_all")
            nc.vector.tensor_copy(
                ms_all[:].rearrange("p r c -> p (r c)"), ms_tp2[:])

        # global M, own rescale factor f_me, global S, 1/S
        Mg = small.tile([128, LT], f32, name="Mg")
        negMg = small.tile([128, LT], f32, name="negMg")
        f_me = small.tile([128, LT], f32, name="f_me")
        Sg = small.tile([128, LT], f32, name="Sg")
        rec = small.tile([128, LT], f32, name="rec")
        for lt in range(LT):
            nc.vector.tensor_reduce(Mg[:, lt:lt + 1], ms_all[:, :, lt:lt + 1],
                                    axis=AX.XY, op=ALU.max)
        nc.vector.tensor_scalar_mul(negMg[:], Mg[:], -1.0)
        for lt in range(LT):
            nc.scalar.activation(f_me[:, lt:lt + 1], m_loc[:, lt:lt + 1],
                                 AF.Exp, bias=negMg[:, lt:lt + 1])
        f_all = small.tile([128, 8, LT], f32, name="f_all")
        for lt in range(LT):
            nc.scalar.activation(f_all[:, :, lt:lt + 1], ms_all[:, :, lt:lt + 1],
                                 AF.Exp, bias=negMg[:, lt:lt + 1])
            nc.vector.tensor_tensor(f_all[:, :, lt:lt + 1], f_all[:, :, lt:lt + 1],
                                    ms_all[:, :, 2 + lt:3 + lt], ALU.mult)
            nc.vector.tensor_reduce(Sg[:, lt:lt + 1], f_all[:, :, lt:lt + 1],
                                    axis=AX.XY, op=ALU.add)
        nc.vector.reciprocal(rec[:], Sg[:])

        # diag(f_me) per l-tile; transpose-and-rescale p via matmul
        diag = small.tile([128, LT, 128], f16, name="diag")
        for lt in range(LT):
            nc.scalar.activation(diag[:, lt, :], ident16[:], AF.Copy,
                                 scale=f_me[:, lt:lt + 1])
        with tc.tile_pool(name="tpps", bufs=2, space="PSUM") as tpps:
            for lt in range(LT):
                for ntl in range(NTL):
                    tp = tpps.tile([128, 128], f32)
                    nc.tensor.matmul(
                        tp[:], p_sb[:, lt, ntl * 128:(ntl + 1) * 128],
                        diag[:, lt, :], start=True, stop=True)
                    nc.vector.tensor_copy(
                        pT_loc[:, ntl, lt * 128:(lt + 1) * 128], tp[:])
        nc.scalar.dma_start(agp_i.ap().rearrange("p (t l) -> p t l", t=NTL),
                            pT_loc[:])
        nc.gpsimd.collective_compute(
            "AllGather", ALU.bypass, replica_groups=RG,
            ins=[agp_i.ap().opt()], outs=[agp_o.ap().opt()])
        pT_4d = pT[:].rearrange("p (r t) l -> p r t l", r=NCORES)
        for c in range(4):
            nc.gpsimd.dma_start(pT_4d[:, c * 2:(c + 1) * 2, :, :],
                                agpo_re[:, c * 2:(c + 1) * 2, :, :])

        # ------------- phase 3: uT[ds_j, L] = x[:, ds_j].T @ pT (full n) -----
        with tc.tile_pool(name="ph3xc", bufs=2) as ph3xc, \
             tc.tile_pool(name="ph3ps", bufs=1, space="PSUM") as ph3ps:
            psu = [ph3ps.tile([128, L], f32, name=f"psu{i}") for i in range(4)]
            for c in range(4):
                xc_c = ph3xc.tile([128, 16, DS], f16, name="xc_c")
                nc.sync.dma_start(xc_c[:], xc_re[:, c * 16:(c + 1) * 16, :])
                for i in range(16):
                    nt = c * 16 + i
                    for ci in range(4):
                        nc.tensor.matmul(
                            psu[ci][:], xc_c[:, i, ci * 128:(ci + 1) * 128],
                            pT[:, nt, :], start=(nt == 0), stop=(nt == NTA - 1))
            for ci in range(4):
                nc.vector.tensor_copy(uT_loc[:, ci, :], psu[ci][:])
        nc.scalar.dma_start(agu_i.ap().rearrange("p (t l) -> p t l", t=4),
                            uT_loc[:])
        nc.gpsimd.collective_compute(
            "AllGather", ALU.bypass, replica_groups=RG,
            ins=[agu_i.ap().opt()], outs=[agu_o.ap().opt()])
        ctxT_4d = ctxT[:].rearrange("p (r t) l -> p r t l", r=NCORES)
        for c in range(4):
            nc.gpsimd.dma_start(ctxT_4d[:, c * 2:(c + 1) * 2, :, :],
                                aguo_re[:, c * 2:(c + 1) * 2, :, :])

        # ------------- phase 4: out = (ctxT/S).T @ WvT + Wv_b ----------------
        with tc.tile_pool(name="ph4wv", bufs=4) as ph4wv, \
             tc.tile_pool(name="ph4ps", bufs=1, space="PSUM") as ph4ps, \
             tc.tile_pool(name="ph4o", bufs=2) as ph4o:
            po = [ph4ps.tile([128, DS], f32, name=f"po{i}") for i in range(LT)]
            for c in range(4):
                wv_c = ph4wv.tile([128, 8, DS], f16, name="wv_c")
                nc.sync.dma_start(wv_c[:], wvt_re[:, c * 8:(c + 1) * 8, :])
                for i in range(8):
                    dt = c * 8 + i
                    for lt in range(LT):
                        nc.tensor.matmul(
                            po[lt][:], ctxT[:, dt, lt * 128:(lt + 1) * 128],
                            wv_c[:, i, :], start=(dt == 0), stop=(dt == DT - 1))
            for lt in range(LT):
                o_sb = ph4o.tile([128, DS], f32)
                nc.scalar.activation(o_sb[:], po[lt][:], AF.Copy,
                                     scale=rec[:, lt:lt + 1])
                nc.vector.tensor_tensor(o_sb[:], o_sb[:], bias_sb[:], ALU.add)
                nc.scalar.dma_start(t_out[lt * 128:(lt + 1) * 128, :], o_sb[:])

    if split_waits:
        _split_waits(nc, mybir, bass_rust)
        _NC = nc
    return nc


def _tile128(a):
    """[T*128, F] -> [128, T*F] so each partition's bytes are contiguous."""
    t = a.shape[0] // 128
    return np.ascontiguousarray(
        a.reshape(t, 128, a.shape[1]).transpose(1, 0, 2).reshape(128, -1))


last_results = None


def kernel(src_prompts, query, Wk_w, Wk_b, Wv_w, Wv_b):
    global last_results
    from concourse.bass_utils import run_bass_kernel_spmd

    nc = _build()

    x = np.asarray(src_prompts, dtype=np.float32)[0]
    q = np.asarray(query, dtype=np.float32)
    wk = np.asarray(Wk_w, dtype=np.float32)
    wv = np.asarray(Wv_w, dtype=np.float32)
    wvb = np.asarray(Wv_b, dtype=np.float32)
    # Wk_b shifts every score row by a constant -> cancels in softmax.

    x16 = x.astype(np.float16)
    qts = _tile128(np.ascontiguousarray(q.T).astype(np.float16))
    in_maps = []
    for j in range(NCORES):
        ns, ds = slice(j * NS, (j + 1) * NS), slice(j * DS, (j + 1) * DS)
        in_maps.append({
            "qts": qts,
            "wk": _tile128(wk[:, ds].astype(np.float16)),
            "xt": _tile128(np.ascontiguousarray(x16[ns].T)),
            "xc": _tile128(np.ascontiguousarray(x16[:, ds])),
            "wvt": _tile128(np.ascontiguousarray(wv[ds].T).astype(np.float16)),
            "wvb": np.ascontiguousarray(wvb[ds][None, :]),
        })

    res = run_bass_kernel_spmd(nc, in_maps, core_ids=list(range(NCORES)))
    last_results = res
    out = np.concatenate([res.results[j]["out"] for j in range(NCORES)], axis=1)
    return out[None, :, :]
